# revision 8
# baseline (speedup 1.0000x reference)
"""Trainium2 Bass kernel for gated multi-head attention (nn_MHAtt_41274635714591).

Strategy: data-parallel over batch — 8 batches onto 8 NeuronCores, one batch per
core, no collectives. Per core (S=1024, D=1024, H=8, DB=128):

  1. Inputs stream as f32 row-tiles; 128x128 transposes on PE in f32
     (2 cycles/row), evicted from PSUM with a fused dtype convert:
     q/k -> fp8e4 xT, v -> bf16 xT. No separate pre-convert pass.
  2. q/k projections run fp8e4 with DoubleRow perf mode (K=256 per matmul,
     0.5 cycles/row): lhsT = W-colblock pair (fp8, x64 scaled), rhs = xT
     subtile pair. Eviction applies x(1/64) and +bias in one DVE op.
     v projection stays bf16 (accuracy-critical path), natural [s, d]
     layout straight into vh_aug whose extra all-ones column yields the
     softmax denominator for free from the PV matmul.
  3. Weight streams ride the ACT DMA queue; x tiles + outputs ride the SP
     queue, so input loads and weight streams overlap from t=0.
  4. Gate MLP for ALL heads (sigmoid directly on ACT); gate rows produced
     already broadcast across partitions by replicating the Wg2 column
     across the matmul's stationary dim; gates multiply khT/qhT in place.
  5. Scores TRANSPOSED: S^T[k,q] = lhsT=khT-chunk, rhs=qhT. exp(scale*x +
     maskbias_k) on ACT writes P^T directly. PV: out[q, 0:129] =
     sum_k P^T-chunk^T @ vh_aug; col 128 = denominator; DVE normalizes.
  6. att tiles transposed on PE into A_T [d, s]; merge matmul with streamed
     Wm col-halves; + bm; DMA out.

The harness calls kernel(**full_inputs); we shard batch across cores with
run_bass_kernel_spmd and stack the per-core outputs.
"""

import math
import os
import sys

for _p in ("/opt/trn_rl_repo", "/root/.axon_site/_ro/trn_rl_repo"):
    if os.path.isdir(_p) and _p not in sys.path:
        sys.path.insert(0, _p)

import numpy as np

import concourse.bass as bass
import concourse.mybir as mybir
import concourse.tile as tile
from concourse import bacc
from concourse.masks import make_identity

F32 = mybir.dt.float32
BF16 = mybir.dt.bfloat16
FP8 = mybir.dt.float8e4
U8 = mybir.dt.uint8
AF = mybir.ActivationFunctionType
OP = mybir.AluOpType
DR = mybir.MatmulPerfMode.DoubleRow

B, S, D, H = 8, 1024, 1024, 8
DB = D // H          # 128 per-head dim
P = 128              # partitions
KJ = S // P          # 8 tiles of 128 along s
NDT = D // P         # 8 tiles of 128 along d
SCALE = 1.0 / math.sqrt(DB)
NEG = -1e9
WS = 64.0            # fp8 weight pre-scale (W*64 keeps sigma~1.3 in e4m3 range)


def build_nc(repeat=1):
    """Emit the per-core program. repeat>1 wraps the body in a device-side
    loop (for timing)."""
    pdt = BF16
    adt = BF16
    nc = bacc.Bacc()

    q = nc.dram_tensor("q", [S, D], F32, kind="ExternalInput")
    k = nc.dram_tensor("k", [S, D], F32, kind="ExternalInput")
    v = nc.dram_tensor("v", [S, D], F32, kind="ExternalInput")
    mask = nc.dram_tensor("mask", [S], U8, kind="ExternalInput")
    Wq = nc.dram_tensor("Wq", [D, D], F32, kind="ExternalInput")
    Wk = nc.dram_tensor("Wk", [D, D], F32, kind="ExternalInput")
    Wv = nc.dram_tensor("Wv", [D, D], F32, kind="ExternalInput")
    Wm = nc.dram_tensor("Wm", [D, D], F32, kind="ExternalInput")
    bq = nc.dram_tensor("bq", [D], F32, kind="ExternalInput")
    bk = nc.dram_tensor("bk", [D], F32, kind="ExternalInput")
    bv = nc.dram_tensor("bv", [D], F32, kind="ExternalInput")
    bm = nc.dram_tensor("bm", [D], F32, kind="ExternalInput")
    WgX = nc.dram_tensor("WgX", [DB, DB], F32, kind="ExternalInput")
    WgY = nc.dram_tensor("WgY", [DB, DB], F32, kind="ExternalInput")
    Wg2 = nc.dram_tensor("Wg2", [DB, 2], F32, kind="ExternalInput")
    bgX = nc.dram_tensor("bgX", [DB], F32, kind="ExternalInput")
    bgY = nc.dram_tensor("bgY", [DB], F32, kind="ExternalInput")
    bg2 = nc.dram_tensor("bg2", [2], F32, kind="ExternalInput")
    out = nc.dram_tensor("out", [S, D], F32, kind="ExternalOutput")

    from contextlib import ExitStack

    with tile.TileContext(nc) as tc, ExitStack() as ctx:
        consts = ctx.enter_context(tc.tile_pool(name="consts", bufs=1))
        persist = ctx.enter_context(tc.tile_pool(name="persist", bufs=1))
        big = ctx.enter_context(tc.tile_pool(name="big", bufs=3))
        xrow = ctx.enter_context(tc.tile_pool(name="xrow", bufs=3))
        wstream = ctx.enter_context(tc.tile_pool(name="wstream", bufs=1))
        wconv = ctx.enter_context(tc.tile_pool(name="wconv", bufs=3))
        gpool = ctx.enter_context(tc.tile_pool(name="gpool", bufs=2))
        attp = ctx.enter_context(tc.tile_pool(name="attp", bufs=2))
        smalls = ctx.enter_context(tc.tile_pool(name="smalls", bufs=2))
        outp = ctx.enter_context(tc.tile_pool(name="outp", bufs=2))
        brep = ctx.enter_context(tc.tile_pool(name="brep", bufs=1))
        # PSUM: psc 2x[128,1024]f32 (4 banks) + ppv 2x[128,129]f32 (2 banks)
        # + ptr [128,1024]f32 (1 buf = 2 banks) = 8 banks
        psc = ctx.enter_context(tc.tile_pool(name="psc", bufs=2, space="PSUM"))
        ppv = ctx.enter_context(tc.tile_pool(name="ppv", bufs=2, space="PSUM"))
        ptr = ctx.enter_context(tc.tile_pool(name="ptr", bufs=2, space="PSUM"))
        if repeat > 1:
            ctx.enter_context(tc.For_i(0, repeat, 1))

        # ---- constants / small prep ----
        identf = consts.tile([P, P], F32, tag="identf")
        make_identity(nc, identf)
        identp = consts.tile([P, P], pdt, tag="identp")
        nc.gpsimd.tensor_copy(identp, identf)

        # Small transposed/broadcast loads go through SWDGE (gpsimd): the
        # HWDGE codegen requires a contiguous fastest-moving dim.
        with nc.allow_non_contiguous_dma(reason="tiny partition-major loads"):
            mask_u8 = consts.tile([P, KJ], U8, tag="mask_u8")
            nc.gpsimd.dma_start(
                out=mask_u8, in_=mask.rearrange("(o p) -> p o", p=P)
            )
            bq_sb = consts.tile([P, NDT], F32, tag="bq_sb")
            nc.gpsimd.dma_start(out=bq_sb, in_=bq.rearrange("(o p) -> p o", p=P))
            bk_sb = consts.tile([P, NDT], F32, tag="bk_sb")
            nc.gpsimd.dma_start(out=bk_sb, in_=bk.rearrange("(o p) -> p o", p=P))
            bgX_sb = consts.tile([P, 1], F32, tag="bgX_sb")
            nc.gpsimd.dma_start(out=bgX_sb, in_=bgX.rearrange("(o p) -> p o", p=P))
            bgY_sb = consts.tile([P, 1], F32, tag="bgY_sb")
            nc.gpsimd.dma_start(out=bgY_sb, in_=bgY.rearrange("(o p) -> p o", p=P))
            # bg2 replicated to every partition (activation bias must be [P, 1])
            bg2r = consts.tile([P, 2], F32, tag="bg2r")
            nc.gpsimd.dma_start(out=bg2r, in_=bg2[None, :].partition_broadcast(P))
            # free-axis bias bv, replicated across partitions (bm shares the
            # slot later — disjoint lifetimes)
            bv_rep = brep.tile([P, D], F32, tag="brep")
            nc.gpsimd.dma_start(out=bv_rep, in_=bv[None, :].partition_broadcast(P))
        maskb = consts.tile([P, KJ], F32, tag="maskb")
        nc.vector.tensor_scalar_mul(maskb, mask_u8, NEG)

        # gate biases as [1,128] rows + all-ones row: bias lands in the gate
        # PSUM via a K=1 rank-1 matmul, so the psums need no separate eviction
        bgX_rf = consts.tile([1, DB], F32, tag="bgX_rf")
        nc.sync.dma_start(out=bgX_rf, in_=bgX[None, :])
        bgY_rf = consts.tile([1, DB], F32, tag="bgY_rf")
        nc.sync.dma_start(out=bgY_rf, in_=bgY[None, :])
        bgX_row = consts.tile([1, DB], adt, tag="bgX_row")
        nc.vector.tensor_copy(bgX_row, bgX_rf)
        bgY_row = consts.tile([1, DB], adt, tag="bgY_row")
        nc.vector.tensor_copy(bgY_row, bgY_rf)
        ones512 = consts.tile([1, 512], adt, tag="ones512")
        nc.vector.memset(ones512, 1.0)

        WgX_f = consts.tile([P, DB], F32, tag="WgX_f")
        nc.sync.dma_start(out=WgX_f, in_=WgX[:, :])
        WgY_f = consts.tile([P, DB], F32, tag="WgY_f")
        nc.sync.dma_start(out=WgY_f, in_=WgY[:, :])
        WgX_sb = consts.tile([P, DB], adt, tag="WgX_sb")
        nc.gpsimd.tensor_copy(WgX_sb, WgX_f)
        WgY_sb = consts.tile([P, DB], adt, tag="WgY_sb")
        nc.gpsimd.tensor_copy(WgY_sb, WgY_f)
        # Wg2 columns replicated across 128 stationary columns: the z matmul
        # then emits each gate row already broadcast over all 128 partitions.
        Wg2_f = consts.tile([P, 2], F32, tag="Wg2_f")
        nc.sync.dma_start(out=Wg2_f, in_=Wg2[:, :])
        Wg2c = consts.tile([P, 2, P], adt, tag="Wg2c")
        nc.vector.tensor_copy(Wg2c, Wg2_f[:, :, None].to_broadcast((P, 2, P)))

        # ---- persistent activations ----
        qhT = persist.tile([P, H, S], adt, tag="qhT")   # [db, h, s] = (q@Wq+b)^T
        khT = persist.tile([P, H, S], adt, tag="khT")
        vh_aug = persist.tile([P, H, KJ, DB + 1], adt, tag="vh_aug")
        nc.vector.memset(vh_aug[:, :, :, DB : DB + 1], 1.0)
        A_T = persist.tile([P, H, S], pdt, tag="A_T")   # attention out, transposed

        # ---- input transpose: x [s, d] f32 -> xT [d-in-tile, i, s] (dtype dt)
        # PE transposes the raw f32 rows (2 cycles/row); the PSUM eviction
        # does the dtype conversion for free (alternating DVE/Pool).
        def load_xT(xdram, dt):
            xT = big.tile([P, NDT, S], dt, tag="bigslab")
            for m in range(KJ):
                xf = xrow.tile([P, D], F32, tag="xrow")
                nc.sync.dma_start(out=xf, in_=xdram[m * P : (m + 1) * P, :])
                for hh in range(2):  # half-tile granularity pipelines PE vs evict
                    pt = ptr.tile([P, 4 * P], F32, tag="trps")
                    for dj in range(4):
                        nc.tensor.transpose(
                            pt[:, dj * P : (dj + 1) * P],
                            xf[:, (hh * 4 + dj) * P : (hh * 4 + dj + 1) * P],
                            identf,
                        )
                    dst = xT[:, hh * 4 : hh * 4 + 4, m * P : (m + 1) * P]
                    src = pt.rearrange("p (a b) -> p a b", b=P)
                    if m % 2 == 0:
                        nc.vector.tensor_copy(dst, src)
                    else:
                        nc.scalar.copy(dst, src)
            return xT

        def load_wch(Wdram, half, dt, scale=None):
            """Stream a [D, 512] column-half of W on the ACT DMA queue,
            converted to dt (optionally pre-scaled). Chunked by pairs of
            128-row blocks so the first matmuls start early."""
            wf = wstream.tile([P, NDT, 512], F32, tag="wch")
            wsrc = Wdram[:, half * 512 : (half + 1) * 512].rearrange(
                "(i p) n -> p i n", p=P
            )
            wb = wconv.tile([P, NDT, 512], dt, tag="wchb", name="wb")
            for c in range(0, NDT, 2):
                nc.scalar.dma_start(out=wf[:, c : c + 2, :], in_=wsrc[:, c : c + 2, :])
                if scale is None:
                    nc.gpsimd.tensor_copy(wb[:, c : c + 2, :], wf[:, c : c + 2, :])
                else:
                    nc.gpsimd.tensor_scalar_mul(
                        wb[:, c : c + 2, :], wf[:, c : c + 2, :], scale
                    )
            return wb

        # ---- q/k projections (fp8 DoubleRow), output transposed [d_out, s] ----
        def proj_T(xT, Wdram, bias_sb, dstT, wch0=None):
            for half in range(2):
                wch = wch0 if (half == 0 and wch0 is not None) else load_wch(
                    Wdram, half, FP8, scale=WS
                )
                for sh in range(2):
                    sl = slice(sh * 512, (sh + 1) * 512)
                    for j4 in range(4):
                        j = half * 4 + j4  # d_out tile == head index
                        ps = psc.tile([P, 512], F32, tag="pacc")
                        for i in range(0, NDT, 2):
                            nc.tensor.matmul(
                                ps,
                                wch[:, i : i + 2, j4 * P : (j4 + 1) * P],
                                xT[:, i : i + 2, sl],
                                start=(i == 0),
                                stop=(i == NDT - 2),
                                perf_mode=DR,
                            )
                        nc.vector.tensor_scalar(
                            dstT[:, j, sl], ps, 1.0 / WS, bias_sb[:, j : j + 1],
                            op0=OP.mult, op1=OP.add,
                        )

        # ---- v projection, natural [s, d_out], + bv, into vh_aug ----
        def proj_v_tile(vT, wch0, wch1, m):
                ps = psc.tile([P, S], F32, tag="pacc")
                for half, wch in ((0, wch0), (1, wch1)):
                    sl = slice(half * 512, (half + 1) * 512)
                    for i in range(NDT):
                        nc.tensor.matmul(
                            ps[:, sl],
                            vT[:, i, m * P : (m + 1) * P],
                            wch[:, i, :],
                            start=(i == 0),
                            stop=(i == NDT - 1),
                        )
                nc.vector.tensor_tensor(
                    vh_aug[:, :, m, 0:DB],
                    ps.rearrange("p (h n) -> p h n", n=DB),
                    bv_rep.rearrange("p (h n) -> p h n", n=DB),
                    OP.add,
                )

        def gates(h):
            gx = gpool.tile([P, S], adt, tag="gx")
            gy = gpool.tile([P, S], adt, tag="gy")
            psx = psc.tile([P, S], F32, tag="pacc")
            for sh in range(2):
                sl = slice(sh * 512, (sh + 1) * 512)
                nc.tensor.matmul(
                    psx[:, sl], WgX_sb, khT[:, h, sl], start=True, stop=True
                )
            nc.scalar.activation(gx, psx, AF.Identity, bias=bgX_sb)
            psy = psc.tile([P, S], F32, tag="pacc")
            for sh in range(2):
                sl = slice(sh * 512, (sh + 1) * 512)
                nc.tensor.matmul(
                    psy[:, sl], WgY_sb, qhT[:, h, sl], start=True, stop=False
                )
                nc.tensor.matmul(
                    psy[:, sl], bgY_row, ones512, start=False, stop=True
                )
            tt = gpool.tile([P, S], adt, tag="tt")
            nc.vector.tensor_tensor(tt, gx, psy, OP.mult)
            # z matmuls with replicated Wg2 columns: every output partition
            # carries the same gate row -> no cross-partition broadcast needed.
            for gi, dstT in ((0, khT), (1, qhT)):
                psz = psc.tile([P, S], F32, tag="pacc")
                for sh in range(2):
                    sl = slice(sh * 512, (sh + 1) * 512)
                    nc.tensor.matmul(
                        psz[:, sl], Wg2c[:, gi, :], tt[:, sl], start=True, stop=True
                    )
                g = gpool.tile([P, S], adt, tag=f"g{gi}")
                nc.scalar.activation(
                    g, psz, AF.Sigmoid, bias=bg2r[:, gi : gi + 1]
                )
                nc.vector.tensor_tensor(dstT[:, h, :], dstT[:, h, :], g, OP.mult)

        # ---- attention helpers (emitted interleaved below) ----
        def scores_exp(h):
            # scores (transposed) + exp -> P^T  [s_k-in-tile, kj, q]
            PT = big.tile([P, KJ, S], adt, tag="bigslab", name="PT")
            for kj in range(KJ):
                ps = psc.tile([P, S], F32, tag="pacc")
                for sh in range(2):
                    sl = slice(sh * 512, (sh + 1) * 512)
                    nc.tensor.matmul(
                        ps[:, sl],
                        khT[:, h, kj * P : (kj + 1) * P],
                        qhT[:, h, sl],
                        start=True,
                        stop=True,
                    )
                nc.scalar.activation(
                    PT[:, kj, :], ps, AF.Exp,
                    bias=maskb[:, kj : kj + 1], scale=SCALE,
                )

            return PT

        def pv_block(h, PT):
            # PV with fused denominator; normalize; transpose into A_T
            for hh in range(2):
                pt2 = ptr.tile([P, 4 * P], pdt, tag="trps")
                for qq in range(4):
                    qi = hh * 4 + qq
                    pv = ppv.tile([P, DB + 1], F32, tag="pv")
                    for kj in range(KJ):
                        nc.tensor.matmul(
                            pv,
                            PT[:, kj, qi * P : (qi + 1) * P],
                            vh_aug[:, h, kj, :],
                            start=(kj == 0),
                            stop=(kj == KJ - 1),
                        )
                    rec = smalls.tile([P, 1], F32, tag="rec")
                    nc.vector.reciprocal(rec, pv[:, DB : DB + 1])
                    asb = attp.tile([P, P], pdt, tag="asb")
                    nc.vector.tensor_scalar_mul(asb, pv[:, 0:DB], rec)
                    nc.tensor.transpose(
                        pt2[:, qq * P : (qq + 1) * P], asb, identp
                    )
                nc.vector.tensor_copy(
                    A_T[:, h, hh * 512 : (hh + 1) * 512], pt2
                )

        # ---- main phase schedule ----
        # q/k: x tiles on SP queue + W streams on ACT queue run concurrently.
        xTq = load_xT(q, FP8)
        wq0 = load_wch(Wq, 0, FP8, scale=WS)
        proj_T(xTq, Wq, bq_sb, qhT, wch0=wq0)
        xTk = load_xT(k, FP8)
        wk0 = load_wch(Wk, 0, FP8, scale=WS)
        proj_T(xTk, Wk, bk_sb, khT, wch0=wk0)

        # v projection with the gate MLP interleaved per s-tile: the gate
        # chains are ACT/DVE-latency-bound, the v matmuls keep PE fed.
        wv0 = load_wch(Wv, 0, pdt)
        wv1 = load_wch(Wv, 1, pdt)
        xTv = load_xT(v, pdt)

        # v-projection with the gate MLP interleaved per s-tile (gate chains
        # are ACT/DVE-latency-bound; v matmuls keep PE fed), and the first two
        # heads' scores pulled into the tail so the gate-chain drain overlaps
        # attention startup. Then attention pipelined one head ahead (exp of
        # h+1 on ACT overlaps PV of h on PE).
        PTs = {}
        for m in range(KJ):
            proj_v_tile(xTv, wv0, wv1, m)
            gates(m)
            if m == 3:
                PTs[0] = scores_exp(0)
            if m == 5:
                PTs[1] = scores_exp(1)
            if m == 7:
                PTs[2] = scores_exp(2)
        for h in range(3, H):
            pv_block(h - 3, PTs.pop(h - 3))
            PTs[h] = scores_exp(h)
        pv_block(H - 3, PTs.pop(H - 3))
        pv_block(H - 2, PTs.pop(H - 2))
        pv_block(H - 1, PTs.pop(H - 1))

        # ---- merge: out = A @ Wm + bm ----
        bm_rep = brep.tile([P, D], F32, tag="brep")
        with nc.allow_non_contiguous_dma(reason="tiny partition-major loads"):
            nc.gpsimd.dma_start(out=bm_rep, in_=bm[None, :].partition_broadcast(P))
        wm0 = load_wch(Wm, 0, pdt)
        wm1 = load_wch(Wm, 1, pdt)
        for m in range(KJ):
            ps = psc.tile([P, S], F32, tag="pacc")
            for half, wch in ((0, wm0), (1, wm1)):
                sl = slice(half * 512, (half + 1) * 512)
                for i in range(NDT):
                    nc.tensor.matmul(
                        ps[:, sl],
                        A_T[:, i, m * P : (m + 1) * P],
                        wch[:, i, :],
                        start=(i == 0),
                        stop=(i == NDT - 1),
                    )
            osb = outp.tile([P, S], F32, tag="osb")
            nc.vector.tensor_tensor(osb, ps, bm_rep, OP.add)
            nc.sync.dma_start(out=out[m * P : (m + 1) * P, :], in_=osb)

    nc.finalize()
    return nc


_NC_CACHE = {}


def _get_nc(key="v1"):
    if key not in _NC_CACHE:
        _NC_CACHE[key] = build_nc()
    return _NC_CACHE[key]


def _f32(a):
    return np.ascontiguousarray(np.asarray(a, dtype=np.float32))


def kernel(v, k, q, mask, Wv, bv, Wk, bk, Wq, bq, Wm, bm,
           WgX, bgX, WgY, bgY, Wg2, bg2):
    from concourse.bass_utils import run_bass_kernel_spmd

    nc = _get_nc()
    nb = int(np.asarray(q).shape[0])
    shared = {
        "Wq": _f32(Wq), "Wk": _f32(Wk), "Wv": _f32(Wv), "Wm": _f32(Wm),
        "bq": _f32(bq), "bk": _f32(bk), "bv": _f32(bv), "bm": _f32(bm),
        "WgX": _f32(WgX), "WgY": _f32(WgY), "Wg2": _f32(Wg2),
        "bgX": _f32(bgX), "bgY": _f32(bgY), "bg2": _f32(bg2),
    }
    in_maps = []
    for b in range(nb):
        m = dict(shared)
        m["q"] = _f32(q[b])
        m["k"] = _f32(k[b])
        m["v"] = _f32(v[b])
        m["mask"] = np.ascontiguousarray(
            np.asarray(mask[b], dtype=np.bool_).reshape(S).view(np.uint8)
        )
        in_maps.append(m)
    res = run_bass_kernel_spmd(nc, in_maps, list(range(nb)))
    return np.stack([res.results[b]["out"] for b in range(nb)]).astype(np.float32)
